# revision 1
# baseline (speedup 1.0000x reference)
"""Trainium2 Bass kernel for CPELayer_ResAG (concept-routed LoRA edit layer).

Computation (per token t with concept c = concept_idx[t]):
    down = edit_direction[t] @ lora_down[c]          # [768]@[768,4] -> [4]
    up   = down @ lora_up[c]                         # [4]@[4,1280]  -> [1280]
    out  = x[t] @ W.T + b_lin + 0.25 * up

Strategy: data-parallel over batch across 8 cores (616 tokens/core).
The routed LoRA is computed densely: A.T[(c,r), t] = lora_down_flat.T @ ed.T
for ALL concepts (only ~6% extra PE work), then masked on-device with a
one-hot built by DVE is_equal (the MoE routing), and contracted back with
lora_up_flat via the tensor engine, accumulating into the same PSUM as the
org matmul.  The bias is folded in as one extra contraction row (ones row in
the masked operand, b_lin row in the lora_up operand).  The 0.25 LoRA scale
is folded into lora_up host-side (exact: power of two).

All host-side work is layout only (transpose / reshape / concat / dtype of
the int indices to f32); every FLOP of the reference runs on device.
"""

import sys
import types

import numpy as np

import concourse.mybir as mybir
import concourse.tile as tile
from concourse import bacc
from concourse.bass_utils import run_bass_kernel_spmd

# If BASS_TRACE is set in the environment, run_bass_kernel_spmd imports
# antenv.axon_hooks, which some containers lack; stub it (None hook ->
# tracing is skipped gracefully, execution unaffected).
try:
    import antenv.axon_hooks  # noqa: F401
except ImportError:
    _m = types.ModuleType("antenv.axon_hooks")
    _m.get_axon_ntff_profile_hook = lambda: None
    _m.set_axon_ntff_profile_hook = lambda h: None
    sys.modules["antenv.axon_hooks"] = _m

# Problem shapes (hardcoded per spec nn_CPELayer_ResAG_19335942766951)
N_CORES = 8
B, T, DIN, DOUT = 64, 77, 768, 1280
N_CONCEPTS, RANK = 50, 4
SCALE = 0.25  # alpha/rank = 1/4, exact power of two
BPC = B // N_CORES          # batches per core = 8
TOK = BPC * T               # tokens per core = 616
NJ = N_CONCEPTS * RANK      # 200 flattened (concept, rank) rows
KJ_PAD = 256                # padded rows: 200 lora + 1 bias + 55 zero
P = 128
KD = DIN // P               # 6 k-tiles of the d_in contraction
NH = 308                    # half of TOK for the A.T psum tiles (>=256 keeps f32r full-rate)
T_EDGES = [0, 128, 256, 384, 512, 616]
N_CHUNKS = [(0, 512), (512, 512), (1024, 256)]

_cache = {}


def _build_bass(mm_dtype, lora_dtype=None):
    nc = bacc.Bacc("TRN2", target_bir_lowering=False, debug=False,
                   num_devices=N_CORES)
    f32 = mybir.dt.float32
    # Tensors consumed by the tensor engine carry the matmul dtype end-to-end
    # (float32r is fp32-layout; the BIR verifier requires producer outputs to
    # be fp32r-typed when a fp32r matmul consumes them).  The LoRA branch
    # (edT/ldT/luB/MT) contributes only ~0.7% of the output scale, so it can
    # run at a lower precision than the org matmul without moving the
    # end-to-end error.
    sdt = mm_dtype
    ldt = lora_dtype if lora_dtype is not None else mm_dtype

    xT_d = nc.dram_tensor("xT", [DIN, TOK], sdt, kind="ExternalInput").ap()
    edT_d = nc.dram_tensor("edT", [DIN, TOK], ldt, kind="ExternalInput").ap()
    idx_d = nc.dram_tensor("idxf", [1, TOK], f32, kind="ExternalInput").ap()
    cv_d = nc.dram_tensor("cvals", [P, 2], f32, kind="ExternalInput").ap()
    WT_d = nc.dram_tensor("WT", [DIN, DOUT], sdt, kind="ExternalInput").ap()
    ldT_d = nc.dram_tensor("ldT", [DIN, NJ], ldt, kind="ExternalInput").ap()
    lu_d = nc.dram_tensor("luB", [KJ_PAD, DOUT], ldt, kind="ExternalInput").ap()
    out_d = nc.dram_tensor("out", [TOK, DOUT], f32, kind="ExternalOutput").ap()

    with tile.TileContext(nc) as tc:
        with (
            tc.tile_pool(name="consts", bufs=1) as consts,
            tc.tile_pool(name="outsb", bufs=5) as outsb,
        ):
            # Load order matters: tiny routing tensors first (masks unblock),
            # then ldT/edT (the A.T chain), then luB (up-matmul rhs) so the
            # early wave-A matmuls can run, and the bulky org operands
            # (xT/WT) last, streaming k-pair by k-pair with the org matmuls
            # tracking their arrival.
            cvals = consts.tile([P, 2], f32, tag="cvals")
            nc.sync.dma_start(cvals[:], cv_d[:, :])

            xT = [None] * KD
            WT = [None] * KD

            def load_kpair(k):
                t_ = consts.tile([P, TOK], sdt, tag=f"xT{k}")
                nc.sync.dma_start(t_[:], xT_d[k * P:(k + 1) * P, :])
                xT[k] = t_
                t_ = consts.tile([P, DOUT], sdt, tag=f"WT{k}")
                nc.sync.dma_start(t_[:], WT_d[k * P:(k + 1) * P, :])
                WT[k] = t_

            # The LoRA-side tensors load as ONE DMA each (3D access pattern,
            # k-tiles side by side in the free dim): 3 sequencer issues
            # instead of 14, so the A.T/MT critical path unblocks ~4us
            # earlier (the small loads were issue-rate bound, not
            # bandwidth bound).
            ld_all = consts.tile([P, KD, NJ], ldt, tag="ld_all")
            nc.sync.dma_start(ld_all[:],
                              ldT_d.rearrange("(k p) j -> p k j", p=P))
            # ed in two halves: the A.T matmuls on k0..2 start while k3..5
            # is still in flight (PE end time = start + busy, so an earlier
            # start is a direct win).
            KH = KD // 2
            ed_a = consts.tile([P, KH, TOK], ldt, tag="ed_a")
            nc.sync.dma_start(ed_a[:],
                              edT_d[0:KH * P, :].rearrange(
                                  "(k p) t -> p k t", p=P))
            ed_b = consts.tile([P, KD - KH, TOK], ldt, tag="ed_b")
            nc.sync.dma_start(ed_b[:],
                              edT_d[KH * P:DIN, :].rearrange(
                                  "(k p) t -> p k t", p=P))
            # Broadcast the token->concept ids across all 128 partitions so a
            # per-partition-scalar is_equal against cvals builds the one-hot.
            idx_bc = consts.tile([P, TOK], f32, tag="idx_bc")
            nc.sync.dma_start(idx_bc[:], idx_d.partition_broadcast(P))

            lu_all = consts.tile([P, 2, DOUT], ldt, tag="lu_all")
            nc.sync.dma_start(lu_all[:],
                              lu_d.rearrange("(j p) o -> p j o", p=P))
            ldT = [ld_all[:, k, :] for k in range(KD)]
            edT = ([ed_a[:, k, :] for k in range(KH)]
                   + [ed_b[:, k, :] for k in range(KD - KH)])
            lu = [lu_all[:, j, :] for j in range(2)]
            for k in range(KD):
                load_kpair(k)

            masks = []
            for jc in range(2):
                m = consts.tile([P, TOK], f32, tag=f"mask{jc}")
                nc.vector.tensor_scalar(
                    m[:], idx_bc[:], cvals[:, jc:jc + 1], None,
                    mybir.AluOpType.is_equal)
                masks.append(m)

            # A.T[(c,r), t] = lora_down_flat.T @ ed.T  for all concepts,
            # masked into MT (the routed "down" activations, transposed).
            MT = []
            for jc in range(2):
                t_ = consts.tile([P, TOK], ldt, tag=f"MT{jc}")
                MT.append(t_)
            # Chunk-1 rows 72..127 pair with luB rows 200..255: engine ops
            # need a 32-aligned start partition, so zero 64..128 first, then
            # the ones row at 96 (bias: b_lin sits at luB[224]); the mask-mul
            # below overwrites rows 0..71 (lora j=128..199).
            # (memset can't target float32r; synthesize 0s/1s via DVE with
            # idx_bc as a donor input, converted on write)
            nc.vector.tensor_scalar(
                MT[1][64:P, :], idx_bc[64:P, :], 0.0, None,
                mybir.AluOpType.mult)
            nc.vector.tensor_scalar(
                MT[1][96:97, :], idx_bc[96:97, :], 0.0, 1.0,
                mybir.AluOpType.mult, mybir.AluOpType.add)

            with tc.tile_pool(name="at_ps", bufs=4, space="PSUM") as at_pool:
                for jc in range(2):
                    jp = P if jc == 0 else NJ - P  # 128, 72
                    jsl = slice(jc * P, jc * P + jp)
                    for nh in range(2):
                        nsl = slice(nh * NH, (nh + 1) * NH)
                        at = at_pool.tile([P, NH], f32, tag="at")
                        for k in range(KD):
                            nc.tensor.matmul(
                                at[:jp, :], ldT[k][:, jsl], edT[k][:, nsl],
                                start=(k == 0), stop=(k == KD - 1))
                        nc.vector.tensor_tensor(
                            MT[jc][:jp, nsl], at[:jp, :], masks[jc][:jp, nsl],
                            mybir.AluOpType.mult)

            # Main accumulation, two short-lived PSUM waves per (t, n) so
            # banks recycle during the load phase instead of every group
            # staying open until the last WT k-tile arrives:
            #   wave A: up1+up2 (MT/lu ready early) + org k0..k2 -> copy osb
            #   wave B: org k3..k5 -> DVE-add into osb
            KA = 3  # org k-tiles in wave A
            with tc.tile_pool(name="out_ps", bufs=8, space="PSUM") as out_pool:
                osbs = []
                for ti in range(len(T_EDGES) - 1):
                    t0, t1 = T_EDGES[ti], T_EDGES[ti + 1]
                    tw = t1 - t0
                    tsl = slice(t0, t1)
                    osb = outsb.tile([P, DOUT], f32, tag="osb")
                    osbs.append(osb)
                    for (n0, nw) in N_CHUNKS:
                        ps = out_pool.tile([P, 512], f32, tag="ops")
                        nmm = 2 + KA
                        i = 0
                        for jc in range(2):
                            nc.tensor.matmul(
                                ps[:tw, :nw], MT[jc][:, tsl],
                                lu[jc][:, n0:n0 + nw],
                                start=(i == 0), stop=(i == nmm - 1))
                            i += 1
                        for k in range(KA):
                            nc.tensor.matmul(
                                ps[:tw, :nw], xT[k][:, tsl],
                                WT[k][:, n0:n0 + nw],
                                start=(i == 0), stop=(i == nmm - 1))
                            i += 1
                        nc.any.tensor_copy(out=osb[:tw, n0:n0 + nw],
                                           in_=ps[:tw, :nw])
                for ti in range(len(T_EDGES) - 1):
                    t0, t1 = T_EDGES[ti], T_EDGES[ti + 1]
                    tw = t1 - t0
                    tsl = slice(t0, t1)
                    osb = osbs[ti]
                    for (n0, nw) in N_CHUNKS:
                        ps = out_pool.tile([P, 512], f32, tag="ops")
                        for i, k in enumerate(range(KA, KD)):
                            nc.tensor.matmul(
                                ps[:tw, :nw], xT[k][:, tsl],
                                WT[k][:, n0:n0 + nw],
                                start=(i == 0), stop=(i == KD - KA - 1))
                        nc.vector.tensor_tensor(
                            osb[:tw, n0:n0 + nw], ps[:tw, :nw],
                            osb[:tw, n0:n0 + nw], mybir.AluOpType.add)
                    nc.sync.dma_start(out_d[tsl, :], osb[:tw, :])

    nc.compile()
    return nc


def get_bass(mm_dtype=None, lora_dtype=None):
    if mm_dtype is None:
        mm_dtype = mybir.dt.float32r
        if lora_dtype is None:
            lora_dtype = mybir.dt.bfloat16
    if lora_dtype is None:
        lora_dtype = mm_dtype
    key = (str(mm_dtype), str(lora_dtype))
    if key not in _cache:
        _cache[key] = _build_bass(mm_dtype, lora_dtype)
    return _cache[key]


def make_in_maps(x, edit_direction, concept_idx, lora_down, lora_up, W, b_lin,
                 np_sdt=np.float32, np_ldt=None):
    """Host-side sharding + layout prep (no reference FLOPs).

    np_sdt: numpy dtype for the org-matmul tensors (xT/WT); np_ldt: dtype
    for the LoRA-branch tensors (edT/ldT/luB), defaults to np_sdt."""
    if np_ldt is None:
        np_ldt = np_sdt
    x = np.asarray(x, dtype=np.float32)
    ed = np.asarray(edit_direction, dtype=np.float32)
    idx = np.asarray(concept_idx)
    ld = np.asarray(lora_down, dtype=np.float32)
    lup = np.asarray(lora_up, dtype=np.float32)
    W = np.asarray(W, dtype=np.float32)
    b = np.asarray(b_lin, dtype=np.float32)

    WT = np.ascontiguousarray(W.T.astype(np_sdt))               # [768, 1280]
    ldT = np.ascontiguousarray(
        ld.transpose(1, 0, 2).reshape(DIN, NJ).astype(np_ldt))
    luB = np.zeros((KJ_PAD, DOUT), dtype=np.float32)
    luB[:NJ] = lup.reshape(NJ, DOUT) * SCALE                    # exact x0.25
    luB[128 + 96] = b                                           # bias row
    luB = luB.astype(np_ldt)
    cv = np.full(2 * P, -1.0, dtype=np.float32)
    cv[:NJ] = np.arange(NJ, dtype=np.float32) // RANK
    cvals = np.ascontiguousarray(cv.reshape(2, P).T)            # [128, 2]

    in_maps = []
    for c in range(N_CORES):
        sl = slice(c * BPC, (c + 1) * BPC)
        xs = x[sl].reshape(TOK, DIN)
        eds = ed[sl].reshape(TOK, DIN)
        idxs = idx[sl].reshape(TOK).astype(np.float32)
        in_maps.append({
            "xT": np.ascontiguousarray(xs.T.astype(np_sdt)),
            "edT": np.ascontiguousarray(eds.T.astype(np_ldt)),
            "idxf": np.ascontiguousarray(idxs.reshape(1, TOK)),
            "cvals": cvals,
            "WT": WT,
            "ldT": ldT,
            "luB": luB,
        })
    return in_maps


def kernel(x, edit_direction, concept_idx, lora_down, lora_up, W, b_lin,
           _trace=False, _mm_dtype=None, _lora_dtype=None):
    if _mm_dtype is None:
        _mm_dtype = mybir.dt.float32r
        if _lora_dtype is None:
            _lora_dtype = mybir.dt.bfloat16
    if _lora_dtype is None:
        _lora_dtype = _mm_dtype
    nc = get_bass(_mm_dtype, _lora_dtype)
    in_maps = make_in_maps(x, edit_direction, concept_idx, lora_down, lora_up,
                           W, b_lin, np_sdt=mybir.dt.np(_mm_dtype),
                           np_ldt=mybir.dt.np(_lora_dtype))
    res = run_bass_kernel_spmd(nc, in_maps, core_ids=list(range(N_CORES)),
                               trace=_trace)
    out = np.concatenate([r["out"] for r in res.results], axis=0)
    out = out.reshape(B, T, DOUT)
    if _trace:
        kernel.last_results = res
    return out



# revision 16
# speedup vs baseline: 1.1346x; 1.1346x over previous
"""Trainium2 Bass kernel for CPELayer_ResAG (concept-routed LoRA edit layer).

Computation (per token t with concept c = concept_idx[t]):
    down = edit_direction[t] @ lora_down[c]          # [768]@[768,4] -> [4]
    up   = down @ lora_up[c]                         # [4]@[4,1280]  -> [1280]
    out  = x[t] @ W.T + b_lin + 0.25 * up

Strategy: data-parallel over batch across 8 cores (616 tokens/core).
The routed LoRA is computed densely: A.T[(c,r), t] = lora_down_flat.T @ ed.T
for ALL concepts, masked on-device with a one-hot built by DVE is_equal (the
MoE routing), and contracted back with lora_up_flat on the tensor engine,
accumulating into the same PSUM as the org matmul.  The bias is folded in as
one extra contraction row (ones row in the masked operand, b_lin row in the
lora_up operand).  The 0.25 LoRA scale is folded into lora_up host-side
(exact: power of two).

Precision/throughput split (rel-err budget 2e-2, achieved ~2.5e-3):
  - org matmul (x@W.T): bf16 operands.  Full PE rate, half the HBM bytes of
    f32r, rel err ~1.7e-3.
  - LoRA branch (ed/ld/lu/MT): fp8e4m3 with DoubleRow matmuls (2 k-tiles per
    instruction at fp8 double-pump rate).  The branch contributes only ~0.7%
    of the output scale, so fp8 error is negligible (~5e-4 end to end).
  - output: bf16 on device, widened to f32 on host (layout-only).

Wave structure per t-block (PSUM banks recycle during the load phase):
  wave A: org k0..k2 -> Act-engine copy to f32 staging
  wave B: up-projection DoubleRow + org k3..k5 -> DVE add -> bf16 out DMA
k-outer ordering inside each wave keeps the PE stationary operand constant
across the 3 n-chunks (fewer weight reloads).
"""

import sys
import types

import numpy as np

import concourse.mybir as mybir
import concourse.tile as tile
from concourse import bacc
from concourse.bass_utils import run_bass_kernel_spmd

# If BASS_TRACE is set in the environment, run_bass_kernel_spmd imports
# antenv.axon_hooks, which some containers lack; stub it (None hook ->
# tracing is skipped gracefully, execution unaffected).
try:
    import antenv.axon_hooks  # noqa: F401
except ImportError:
    _m = types.ModuleType("antenv.axon_hooks")
    _m.get_axon_ntff_profile_hook = lambda: None
    _m.set_axon_ntff_profile_hook = lambda h: None
    sys.modules["antenv.axon_hooks"] = _m

# Problem shapes (hardcoded per spec nn_CPELayer_ResAG_19335942766951)
N_CORES = 8
B, T, DIN, DOUT = 64, 77, 768, 1280
N_CONCEPTS, RANK = 50, 4
SCALE = 0.25  # alpha/rank = 1/4, exact power of two
BPC = B // N_CORES          # batches per core = 8
TOK = BPC * T               # tokens per core = 616
TOKP = 640                  # padded tokens (DoubleRow stationaries need
                            # 128-wide blocks; pad cols are masked/discarded)
NJ = N_CONCEPTS * RANK      # 200 flattened (concept, rank) rows
KJ_PAD = 256                # padded rows: 200 lora + 1 bias + 55 zero
P = 128
KD = DIN // P               # 6 k-tiles of the d_in contraction
KA = 3                      # org k-tiles in wave A (k0..k2); wave B = k3..k5
T_EDGES = [0, 128, 256, 384, 512, 640]
N_CHUNKS = [(0, 512), (512, 512), (1024, 256)]
# A.T token chunks (DoubleRow moving free <= 512 => <=256 tokens per chunk),
# grouped into two PSUM tiles per concept j-tile: [256+256 | 128]
AT_TILES = [[(0, 256), (256, 256)], [(512, 128)]]

_cache = {}


def _build_bass():
    nc = bacc.Bacc("TRN2", target_bir_lowering=False, debug=False,
                   num_devices=N_CORES)
    f32 = mybir.dt.float32
    bf16 = mybir.dt.bfloat16
    fp8 = mybir.dt.float8e4

    xT_d = nc.dram_tensor("xT", [DIN, TOKP], bf16, kind="ExternalInput").ap()
    edT_d = nc.dram_tensor("edT", [DIN, TOKP], fp8, kind="ExternalInput").ap()
    idx_d = nc.dram_tensor("idxf", [1, TOKP], f32, kind="ExternalInput").ap()
    cv_d = nc.dram_tensor("cvals", [P, 2], f32, kind="ExternalInput").ap()
    WT_d = nc.dram_tensor("WT", [DIN, DOUT], bf16, kind="ExternalInput").ap()
    ldT_d = nc.dram_tensor("ldT", [DIN, KJ_PAD], fp8,
                           kind="ExternalInput").ap()
    lu_d = nc.dram_tensor("luB", [KJ_PAD, DOUT], fp8, kind="ExternalInput").ap()
    out_d = nc.dram_tensor("out", [TOKP, DOUT], bf16, kind="ExternalOutput").ap()

    with tile.TileContext(nc) as tc:
        with (
            tc.tile_pool(name="consts", bufs=1) as consts,
            tc.tile_pool(name="outsb", bufs=10) as outsb,
        ):
            # Load order: tiny routing tensors first (masks unblock), then
            # ld/ed pairs (the A.T chain), lu (up-matmul rhs), and the bulky
            # org operands (xT/WT) last, streaming k-pair by k-pair with the
            # org matmuls tracking their arrival.
            cvals = consts.tile([P, 2], f32, tag="cvals")
            nc.sync.dma_start(cvals[:], cv_d[:, :])
            idx_bc = consts.tile([P, TOKP], f32, tag="idx_bc")
            nc.sync.dma_start(idx_bc[:], idx_d.partition_broadcast(P))

            # ld per concept j-chunk so DoubleRow k-pair slices are
            # contiguous (ISA requires pair stride == stationary free width)
            # and 128 columns wide (zero-padded past row 199 host-side).
            ld_jc = []
            for jc in range(2):
                t_ = consts.tile([P, KD, P], fp8, tag=f"ld{jc}")
                nc.sync.dma_start(t_[:],
                                  ldT_d[:, jc * P:(jc + 1) * P].rearrange(
                                      "(k p) j -> p k j", p=P))
                ld_jc.append(t_)
            # ed in 3 DoubleRow k-pair tiles: the A.T matmuls on pair 0 start
            # while pairs 1-2 are still in flight.
            ed_p = []
            for i in range(KD // 2):
                t_ = consts.tile([P, 2, TOKP], fp8, tag=f"ed_p{i}")
                nc.sync.dma_start(t_[:],
                                  edT_d[i * 2 * P:(i + 1) * 2 * P, :].rearrange(
                                      "(k p) t -> p k t", p=P))
                ed_p.append(t_)
            lu_all = consts.tile([P, 2, DOUT], fp8, tag="lu_all")
            nc.sync.dma_start(lu_all[:],
                              lu_d.rearrange("(j p) o -> p j o", p=P))

            xT = [None] * KD
            WT = [None] * KD
            for k in range(KD):
                t_ = consts.tile([P, TOKP], bf16, tag=f"xT{k}")
                nc.sync.dma_start(t_[:], xT_d[k * P:(k + 1) * P, :])
                xT[k] = t_
                t_ = consts.tile([P, DOUT], bf16, tag=f"WT{k}")
                nc.sync.dma_start(t_[:], WT_d[k * P:(k + 1) * P, :])
                WT[k] = t_

            masks = []
            for jc in range(2):
                m = consts.tile([P, TOKP], f32, tag=f"mask{jc}")
                nc.vector.tensor_scalar(
                    m[:], idx_bc[:], cvals[:, jc:jc + 1], None,
                    mybir.AluOpType.is_equal)
                masks.append(m)

            # MT[(c,r) row, j-tile, t]: the routed "down" activations,
            # transposed, in fp8 for the DoubleRow up-projection.  One tile
            # per t-block so the stationary k-pair slice is contiguous.
            # j-tile 1 rows 72..127 pair with luB rows 200..255: zero them,
            # then the ones row at 96 (bias: b_lin sits at luB[224]); the
            # mask-mul below overwrites rows 0..71 (lora j=128..199).
            MTb = []
            for ti in range(len(T_EDGES) - 1):
                tw = T_EDGES[ti + 1] - T_EDGES[ti]
                mt = consts.tile([P, 2, tw], fp8, tag=f"MT{ti}")
                nc.vector.memset(mt[64:P, 1, :], 0.0)
                nc.vector.memset(mt[96:97, 1, :], 1.0)
                MTb.append(mt)

            # A.T[(c,r), t] = lora_down_flat.T @ ed.T for all concepts via
            # fp8 DoubleRow (2 k-tiles per matmul), masked into MT.
            with tc.tile_pool(name="at_ps", bufs=4, space="PSUM") as at_pool:
                for jc in range(2):
                    # matmul always 128 rows (ld zero-padded); the mask-mul
                    # only writes the 72 real lora rows of j-chunk 1 so the
                    # memset bias/zero rows survive.
                    mjp = P if jc == 0 else NJ - P  # 128, 72
                    for chunks in AT_TILES:
                        at = at_pool.tile([P, 512], f32, tag="at")
                        base = chunks[0][0]
                        ni = 0
                        nmm = len(chunks) * (KD // 2)
                        for (n0, nw) in chunks:
                            for i in range(KD // 2):
                                nc.tensor.matmul(
                                    at[:, n0 - base:n0 - base + nw],
                                    ld_jc[jc][:, 2 * i:2 * i + 2, :],
                                    ed_p[i][:, :, n0:n0 + nw],
                                    start=(ni == 0), stop=(ni == nmm - 1),
                                    perf_mode=mybir.MatmulPerfMode.DoubleRow)
                                ni += 1
                        cw = sum(nw for _, nw in chunks)
                        # scatter the masked rows into the per-t-block MT
                        # tiles covered by this psum tile ([0:512] spans
                        # t-blocks 0-3; [512:616] is exactly t-block 4)
                        for ti in range(len(T_EDGES) - 1):
                            t0, t1 = T_EDGES[ti], T_EDGES[ti + 1]
                            if t0 < base or t1 > base + cw:
                                continue
                            nc.vector.tensor_tensor(
                                MTb[ti][:mjp, jc, :],
                                at[:mjp, t0 - base:t1 - base],
                                masks[jc][:mjp, t0:t1],
                                mybir.AluOpType.mult)

            # Main accumulation, two short-lived PSUM waves per (t, n) so
            # banks recycle during the load phase:
            #   wave A: org k0..k2 (x/W k-tiles arrive first) -> copy to f32
            #           staging on the Act engine
            #   wave B: up-projection DoubleRow (MT/lu ready by then) +
            #           org k3..k5 -> DVE add -> bf16 out DMA
            with tc.tile_pool(name="out_ps", bufs=8, space="PSUM") as out_pool:
                osbs = []
                for ti in range(len(T_EDGES) - 1):
                    t0, t1 = T_EDGES[ti], T_EDGES[ti + 1]
                    tw = t1 - t0
                    tsl = slice(t0, t1)
                    osb = outsb.tile([P, DOUT], f32, tag="osb")
                    osbs.append(osb)
                    pss = []
                    for _ci in range(len(N_CHUNKS)):
                        ps = out_pool.tile([P, 512], f32, tag="ops")
                        pss.append(ps)
                    for ki, k in enumerate(range(KA)):
                        for ci, (n0, nw) in enumerate(N_CHUNKS):
                            nc.tensor.matmul(
                                pss[ci][:tw, :nw], xT[k][:, tsl],
                                WT[k][:, n0:n0 + nw],
                                start=(ki == 0), stop=(ki == KA - 1))
                    for ci, (n0, nw) in enumerate(N_CHUNKS):
                        nc.any.tensor_copy(out=osb[:tw, n0:n0 + nw],
                                           in_=pss[ci][:tw, :nw])
                for ti in range(len(T_EDGES) - 1):
                    t0, t1 = T_EDGES[ti], T_EDGES[ti + 1]
                    tw = t1 - t0
                    tsl = slice(t0, t1)
                    osb = osbs[ti]
                    obb = outsb.tile([P, DOUT], bf16, tag="obb")
                    pss = []
                    for _ci in range(len(N_CHUNKS)):
                        ps = out_pool.tile([P, 512], f32, tag="ops")
                        pss.append(ps)
                    # up-projection first: one DoubleRow per 256-wide half
                    # (contraction = both concept j-tiles at once).  First
                    # instr per PSUM tile starts the group; later halves
                    # land in the pending-zero region.
                    for ci, (n0, nw) in enumerate(N_CHUNKS):
                        for h0 in range(0, nw, 256):
                            nc.tensor.matmul(
                                pss[ci][:tw, h0:h0 + 256],
                                MTb[ti][:, :, :],
                                lu_all[:, :, n0 + h0:n0 + h0 + 256],
                                start=(h0 == 0), stop=False,
                                perf_mode=mybir.MatmulPerfMode.DoubleRow)
                    for ki, k in enumerate(range(KA, KD)):
                        for ci, (n0, nw) in enumerate(N_CHUNKS):
                            nc.tensor.matmul(
                                pss[ci][:tw, :nw], xT[k][:, tsl],
                                WT[k][:, n0:n0 + nw],
                                start=False, stop=(k == KD - 1))
                    for ci, (n0, nw) in enumerate(N_CHUNKS):
                        nc.vector.tensor_tensor(
                            obb[:tw, n0:n0 + nw], pss[ci][:tw, :nw],
                            osb[:tw, n0:n0 + nw], mybir.AluOpType.add)
                    nc.sync.dma_start(out_d[tsl, :], obb[:tw, :])

    nc.compile()
    return nc


def get_bass():
    if "nc" not in _cache:
        _cache["nc"] = _build_bass()
    return _cache["nc"]


def make_in_maps(x, edit_direction, concept_idx, lora_down, lora_up, W, b_lin):
    """Host-side sharding + layout/dtype prep (no reference FLOPs)."""
    import ml_dtypes
    bf16 = ml_dtypes.bfloat16
    fp8 = ml_dtypes.float8_e4m3

    x = np.asarray(x, dtype=np.float32)
    ed = np.asarray(edit_direction, dtype=np.float32)
    idx = np.asarray(concept_idx)
    ld = np.asarray(lora_down, dtype=np.float32)
    lup = np.asarray(lora_up, dtype=np.float32)
    W = np.asarray(W, dtype=np.float32)
    b = np.asarray(b_lin, dtype=np.float32)

    WT = np.ascontiguousarray(W.T.astype(bf16))                 # [768, 1280]
    ldT = np.zeros((DIN, KJ_PAD), dtype=np.float32)
    ldT[:, :NJ] = ld.transpose(1, 0, 2).reshape(DIN, NJ)
    ldT = np.ascontiguousarray(ldT.astype(fp8))
    luB = np.zeros((KJ_PAD, DOUT), dtype=np.float32)
    luB[:NJ] = lup.reshape(NJ, DOUT) * SCALE                    # exact x0.25
    luB[128 + 96] = b                                           # bias row
    luB = luB.astype(fp8)
    cv = np.full(2 * P, -1.0, dtype=np.float32)
    cv[:NJ] = np.arange(NJ, dtype=np.float32) // RANK
    cvals = np.ascontiguousarray(cv.reshape(2, P).T)            # [128, 2]

    in_maps = []
    for c in range(N_CORES):
        sl = slice(c * BPC, (c + 1) * BPC)
        xs = np.zeros((TOKP, DIN), dtype=np.float32)
        xs[:TOK] = x[sl].reshape(TOK, DIN)
        eds = np.zeros((TOKP, DIN), dtype=np.float32)
        eds[:TOK] = ed[sl].reshape(TOK, DIN)
        idxs = np.full(TOKP, -1.0, dtype=np.float32)
        idxs[:TOK] = idx[sl].reshape(TOK).astype(np.float32)
        in_maps.append({
            "xT": np.ascontiguousarray(xs.T.astype(bf16)),
            "edT": np.ascontiguousarray(eds.T.astype(fp8)),
            "idxf": np.ascontiguousarray(idxs.reshape(1, TOKP)),
            "cvals": cvals,
            "WT": WT,
            "ldT": ldT,
            "luB": luB,
        })
    return in_maps


def kernel(x, edit_direction, concept_idx, lora_down, lora_up, W, b_lin,
           _trace=False, **_ignored):
    nc = get_bass()
    in_maps = make_in_maps(x, edit_direction, concept_idx, lora_down, lora_up,
                           W, b_lin)
    res = run_bass_kernel_spmd(nc, in_maps, core_ids=list(range(N_CORES)),
                               trace=_trace)
    out = np.concatenate([np.asarray(r["out"][:TOK], dtype=np.float32)
                          for r in res.results], axis=0)
    out = out.reshape(B, T, DOUT)
    if _trace:
        kernel.last_results = res
    return out


# revision 17
# speedup vs baseline: 1.2664x; 1.1162x over previous
"""Trainium2 Bass kernel for CPELayer_ResAG (concept-routed LoRA edit layer).

Computation (per token t with concept c = concept_idx[t]):
    down = edit_direction[t] @ lora_down[c]          # [768]@[768,4] -> [4]
    up   = down @ lora_up[c]                         # [4]@[4,1280]  -> [1280]
    out  = x[t] @ W.T + b_lin + 0.25 * up

Strategy: data-parallel over batch across 8 cores (616 tokens/core, padded
to 640 so every block is 128 wide).  The routed LoRA is computed densely:
A.T[(c,r), t] = lora_down_flat.T @ ed.T for ALL concepts, masked on-device
with a one-hot built by DVE is_equal (the MoE routing), and contracted back
with lora_up_flat on the tensor engine, accumulating into the same PSUM as
the org matmul.  The bias is folded in as one extra contraction row (ones
row in the masked operand, b_lin row in the lora_up operand).  The 0.25
LoRA scale is folded into lora_up host-side (exact: power of two).

Precision/throughput split (rel-err budget 2e-2, achieved ~1.7e-2):
  - org matmul k-tiles 0-1 (256 of 768 contraction rows): fp8e4m3 with one
    DoubleRow matmul per n-chunk half -- 2x PE rate and half the bytes.
  - org k-tiles 2-5: bf16 operands.
  - LoRA branch (ed/ld/lu/MT): fp8e4m3 DoubleRow throughout (the branch is
    ~0.7% of the output scale; fp8 error there is negligible).
  - output: bf16 on device, widened to f32 on host (layout-only).

All fp8 DoubleRow stationaries are 128 columns wide with k-pairs contiguous
in SBUF (ISA requirement); the host pre-swizzles every operand so each DMA
is a plain [128 x contiguous-bytes] copy (the strided-gather DMAs of the
previous revision ran at ~270 GB/s; plain rows run at ~400 GB/s).

Wave structure per t-block (PSUM banks recycle during the load phase):
  wave A: org k0k1 DoubleRow + k2,k3 bf16 -> copy to f32 staging
  wave B: up-projection DoubleRow + k4,k5 bf16 -> DVE add -> bf16 out DMA
k-outer ordering inside each wave keeps the PE stationary operand constant
across the n-chunks (fewer weight reloads).
"""

import sys
import types

import numpy as np

import concourse.mybir as mybir
import concourse.tile as tile
from concourse import bacc
from concourse.bass_utils import run_bass_kernel_spmd

# If BASS_TRACE is set in the environment, run_bass_kernel_spmd imports
# antenv.axon_hooks, which some containers lack; stub it (None hook ->
# tracing is skipped gracefully, execution unaffected).
try:
    import antenv.axon_hooks  # noqa: F401
except ImportError:
    _m = types.ModuleType("antenv.axon_hooks")
    _m.get_axon_ntff_profile_hook = lambda: None
    _m.set_axon_ntff_profile_hook = lambda h: None
    sys.modules["antenv.axon_hooks"] = _m

# Problem shapes (hardcoded per spec nn_CPELayer_ResAG_19335942766951)
N_CORES = 8
B, T, DIN, DOUT = 64, 77, 768, 1280
N_CONCEPTS, RANK = 50, 4
SCALE = 0.25  # alpha/rank = 1/4, exact power of two
BPC = B // N_CORES          # batches per core = 8
TOK = BPC * T               # tokens per core = 616
TOKP = 640                  # padded tokens (DoubleRow stationaries need
                            # 128-wide blocks; pad cols are masked/discarded)
NJ = N_CONCEPTS * RANK      # 200 flattened (concept, rank) rows
KJ_PAD = 256                # padded rows: 200 lora + 1 bias + 55 zero
P = 128
KD = DIN // P               # 6 k-tiles of the d_in contraction
KF = 2                      # leading k-tiles in fp8 DoubleRow (k0,k1)
NT = TOKP // P              # 5 t-blocks of 128 tokens
KP = KD // 2                # 3 DoubleRow k-pairs for the A.T matmul
N_CHUNKS = [(0, 512), (512, 512), (1024, 256)]
# A.T token chunks (DoubleRow moving free <= 512 => <=256 tokens per chunk),
# grouped into two PSUM tiles per concept j-tile: [256+256 | 128]
AT_TILES = [[(0, 256), (256, 256)], [(512, 128)]]

_cache = {}


def _build_bass():
    nc = bacc.Bacc("TRN2", target_bir_lowering=False, debug=False,
                   num_devices=N_CORES)
    f32 = mybir.dt.float32
    bf16 = mybir.dt.bfloat16
    fp8 = mybir.dt.float8e4

    # Pre-swizzled host layouts: every tensor is [128, row-bytes] with the
    # SBUF tile's free dims flattened along the row.
    idx_d = nc.dram_tensor("idxf", [1, TOKP], f32, kind="ExternalInput").ap()
    cv_d = nc.dram_tensor("cvals", [P, 2], f32, kind="ExternalInput").ap()
    ld_d = nc.dram_tensor("ldH", [P, 2 * KD * P], fp8,
                          kind="ExternalInput").ap()
    ed_d = nc.dram_tensor("edH", [P, KP * 2 * TOKP], fp8,
                          kind="ExternalInput").ap()
    x8_d = nc.dram_tensor("x8H", [P, NT * KF * P], fp8,
                          kind="ExternalInput").ap()
    W8_d = nc.dram_tensor("W8H", [P, KF * DOUT], fp8,
                          kind="ExternalInput").ap()
    lu_d = nc.dram_tensor("luH", [P, 2 * DOUT], fp8,
                          kind="ExternalInput").ap()
    xT_d = nc.dram_tensor("xT", [DIN, TOKP], bf16, kind="ExternalInput").ap()
    WT_d = nc.dram_tensor("WT", [DIN, DOUT], bf16, kind="ExternalInput").ap()
    out_d = nc.dram_tensor("out", [TOKP, DOUT], bf16,
                           kind="ExternalOutput").ap()

    with tile.TileContext(nc) as tc:
        with (
            tc.tile_pool(name="consts", bufs=1) as consts,
            tc.tile_pool(name="outsb", bufs=10) as outsb,
        ):
            # Load order tracks the consumption order: the A.T chain (ld/ed)
            # first, then the wave-A org operands (x8/W8, xT/WT k2-3), the
            # up-matmul rhs (lu), and the wave-B org tail (k4-5).
            cvals = consts.tile([P, 2], f32, tag="cvals")
            nc.sync.dma_start(cvals[:], cv_d[:, :])
            ld_jc = []
            for jc in range(2):
                t_ = consts.tile([P, KD, P], fp8, tag=f"ld{jc}")
                nc.sync.dma_start(t_[:],
                                  ld_d[:, jc * KD * P:(jc + 1) * KD * P]
                                  .rearrange("p (k j) -> p k j", k=KD))
                ld_jc.append(t_)
            ed_all = consts.tile([P, KP, 2, TOKP], fp8, tag="ed_all")
            nc.sync.dma_start(ed_all[:],
                              ed_d.rearrange("p (i h t) -> p i h t",
                                             i=KP, h=2))
            idx_bc = consts.tile([P, TOKP], f32, tag="idx_bc")
            nc.sync.dma_start(idx_bc[:], idx_d.partition_broadcast(P))

            x8 = consts.tile([P, NT, KF, P], fp8, tag="x8")
            nc.sync.dma_start(x8[:],
                              x8_d.rearrange("p (t h u) -> p t h u",
                                             t=NT, h=KF))
            W8 = consts.tile([P, KF, DOUT], fp8, tag="W8")
            nc.sync.dma_start(W8[:],
                              W8_d.rearrange("p (h o) -> p h o", h=KF))

            xT = {}
            WT = {}

            def load_kpair(k):
                t_ = consts.tile([P, TOKP], bf16, tag=f"xT{k}")
                nc.sync.dma_start(t_[:], xT_d[k * P:(k + 1) * P, :])
                xT[k] = t_
                t_ = consts.tile([P, DOUT], bf16, tag=f"WT{k}")
                nc.sync.dma_start(t_[:], WT_d[k * P:(k + 1) * P, :])
                WT[k] = t_

            load_kpair(2)
            load_kpair(3)
            lu_all = consts.tile([P, 2, DOUT], fp8, tag="lu_all")
            nc.sync.dma_start(lu_all[:],
                              lu_d.rearrange("p (j o) -> p j o", j=2))
            load_kpair(4)
            load_kpair(5)

            masks = []
            for jc in range(2):
                m = consts.tile([P, TOKP], f32, tag=f"mask{jc}")
                nc.vector.tensor_scalar(
                    m[:], idx_bc[:], cvals[:, jc:jc + 1], None,
                    mybir.AluOpType.is_equal)
                masks.append(m)

            # MT[(c,r) row, j-tile, t]: the routed "down" activations,
            # transposed, in fp8 for the DoubleRow up-projection.  One tile
            # per t-block so the stationary k-pair slice is contiguous.
            # j-tile 1 rows 72..127 pair with luB rows 200..255: zero them,
            # then the ones row at 96 (bias: b_lin sits at luB[224]); the
            # mask-mul below overwrites rows 0..71 (lora j=128..199).
            MTb = []
            for ti in range(NT):
                mt = consts.tile([P, 2, P], fp8, tag=f"MT{ti}")
                nc.vector.memset(mt[64:P, 1, :], 0.0)
                nc.vector.memset(mt[96:97, 1, :], 1.0)
                MTb.append(mt)

            # A.T[(c,r), t] = lora_down_flat.T @ ed.T for all concepts via
            # fp8 DoubleRow (2 k-tiles per matmul), masked into MT.
            with tc.tile_pool(name="at_ps", bufs=4, space="PSUM") as at_pool:
                for jc in range(2):
                    # matmul always 128 rows (ld zero-padded); the mask-mul
                    # only writes the 72 real lora rows of j-chunk 1 so the
                    # memset bias/zero rows survive.
                    mjp = P if jc == 0 else NJ - P  # 128, 72
                    for chunks in AT_TILES:
                        at = at_pool.tile([P, 512], f32, tag="at")
                        base = chunks[0][0]
                        ni = 0
                        nmm = len(chunks) * KP
                        for (n0, nw) in chunks:
                            for i in range(KP):
                                nc.tensor.matmul(
                                    at[:, n0 - base:n0 - base + nw],
                                    ld_jc[jc][:, 2 * i:2 * i + 2, :],
                                    ed_all[:, i, :, n0:n0 + nw],
                                    start=(ni == 0), stop=(ni == nmm - 1),
                                    perf_mode=mybir.MatmulPerfMode.DoubleRow)
                                ni += 1
                        cw = sum(nw for _, nw in chunks)
                        # scatter the masked rows into the per-t-block MT
                        # tiles covered by this psum tile ([0:512] spans
                        # t-blocks 0-3; [512:640] is exactly t-block 4)
                        for ti in range(NT):
                            t0, t1 = ti * P, (ti + 1) * P
                            if t0 < base or t1 > base + cw:
                                continue
                            nc.vector.tensor_tensor(
                                MTb[ti][:mjp, jc, :],
                                at[:mjp, t0 - base:t1 - base],
                                masks[jc][:mjp, t0:t1],
                                mybir.AluOpType.mult)

            # Main accumulation, two short-lived PSUM waves per (t, n) so
            # banks recycle during the load phase:
            #   wave A: org k0k1 (one fp8 DoubleRow per 256-half) + k2,k3
            #           bf16 -> copy to f32 staging
            #   wave B: up-projection DoubleRow + org k4,k5 -> DVE add ->
            #           bf16 out DMA
            with tc.tile_pool(name="out_ps", bufs=8, space="PSUM") as out_pool:
                osbs = []
                for ti in range(NT):
                    t0 = ti * P
                    tsl = slice(t0, t0 + P)
                    osb = outsb.tile([P, DOUT], f32, tag="osb")
                    osbs.append(osb)
                    pss = []
                    for _ci in range(len(N_CHUNKS)):
                        ps = out_pool.tile([P, 512], f32, tag="ops")
                        pss.append(ps)
                    for ci, (n0, nw) in enumerate(N_CHUNKS):
                        for h0 in range(0, nw, 256):
                            nc.tensor.matmul(
                                pss[ci][:, h0:h0 + 256],
                                x8[:, ti, :, :],
                                W8[:, :, n0 + h0:n0 + h0 + 256],
                                start=(h0 == 0), stop=False,
                                perf_mode=mybir.MatmulPerfMode.DoubleRow)
                    for k in (2, 3):
                        for ci, (n0, nw) in enumerate(N_CHUNKS):
                            nc.tensor.matmul(
                                pss[ci][:, :nw], xT[k][:, tsl],
                                WT[k][:, n0:n0 + nw],
                                start=False, stop=(k == 3))
                    for ci, (n0, nw) in enumerate(N_CHUNKS):
                        nc.any.tensor_copy(out=osb[:, n0:n0 + nw],
                                           in_=pss[ci][:, :nw])
                for ti in range(NT):
                    t0 = ti * P
                    tsl = slice(t0, t0 + P)
                    osb = osbs[ti]
                    obb = outsb.tile([P, DOUT], bf16, tag="obb")
                    pss = []
                    for _ci in range(len(N_CHUNKS)):
                        ps = out_pool.tile([P, 512], f32, tag="ops")
                        pss.append(ps)
                    for ci, (n0, nw) in enumerate(N_CHUNKS):
                        for h0 in range(0, nw, 256):
                            nc.tensor.matmul(
                                pss[ci][:, h0:h0 + 256],
                                MTb[ti][:, :, :],
                                lu_all[:, :, n0 + h0:n0 + h0 + 256],
                                start=(h0 == 0), stop=False,
                                perf_mode=mybir.MatmulPerfMode.DoubleRow)
                    for k in (4, 5):
                        for ci, (n0, nw) in enumerate(N_CHUNKS):
                            nc.tensor.matmul(
                                pss[ci][:, :nw], xT[k][:, tsl],
                                WT[k][:, n0:n0 + nw],
                                start=False, stop=(k == 5))
                    for ci, (n0, nw) in enumerate(N_CHUNKS):
                        nc.vector.tensor_tensor(
                            obb[:, n0:n0 + nw], pss[ci][:, :nw],
                            osb[:, n0:n0 + nw], mybir.AluOpType.add)
                    nc.sync.dma_start(out_d[tsl, :], obb[:, :])

    nc.compile()
    return nc


def get_bass():
    if "nc" not in _cache:
        _cache["nc"] = _build_bass()
    return _cache["nc"]


def make_in_maps(x, edit_direction, concept_idx, lora_down, lora_up, W, b_lin):
    """Host-side sharding + layout/dtype prep (no reference FLOPs)."""
    import ml_dtypes
    bf16 = ml_dtypes.bfloat16
    fp8 = ml_dtypes.float8_e4m3

    x = np.asarray(x, dtype=np.float32)
    ed = np.asarray(edit_direction, dtype=np.float32)
    idx = np.asarray(concept_idx)
    ld = np.asarray(lora_down, dtype=np.float32)
    lup = np.asarray(lora_up, dtype=np.float32)
    W = np.asarray(W, dtype=np.float32)
    b = np.asarray(b_lin, dtype=np.float32)

    WTf = np.ascontiguousarray(W.T)                             # [768, 1280]
    WT = WTf.astype(bf16)
    # W8H[p, h*1280+o] = W.T[h*128+p, o] for k-tiles h in {0,1}
    W8H = np.ascontiguousarray(
        WTf[:KF * P].reshape(KF, P, DOUT).transpose(1, 0, 2)
        .reshape(P, KF * DOUT).astype(fp8))

    # ldH[p, jc*768 + k*128 + j] = lora_down_flat[k*128+p, jc*128+j]
    ldT = np.zeros((DIN, KJ_PAD), dtype=np.float32)
    ldT[:, :NJ] = ld.transpose(1, 0, 2).reshape(DIN, NJ)
    ldH = np.ascontiguousarray(
        ldT.reshape(KD, P, 2, P).transpose(1, 2, 0, 3)
        .reshape(P, 2 * KD * P).astype(fp8))

    # luH[p, j*1280+o] = luB[j*128+p, o]
    luB = np.zeros((KJ_PAD, DOUT), dtype=np.float32)
    luB[:NJ] = lup.reshape(NJ, DOUT) * SCALE                    # exact x0.25
    luB[128 + 96] = b                                           # bias row
    luH = np.ascontiguousarray(
        luB.reshape(2, P, DOUT).transpose(1, 0, 2)
        .reshape(P, 2 * DOUT).astype(fp8))

    cv = np.full(2 * P, -1.0, dtype=np.float32)
    cv[:NJ] = np.arange(NJ, dtype=np.float32) // RANK
    cvals = np.ascontiguousarray(cv.reshape(2, P).T)            # [128, 2]

    in_maps = []
    for c in range(N_CORES):
        sl = slice(c * BPC, (c + 1) * BPC)
        xs = np.zeros((TOKP, DIN), dtype=np.float32)
        xs[:TOK] = x[sl].reshape(TOK, DIN)
        eds = np.zeros((TOKP, DIN), dtype=np.float32)
        eds[:TOK] = ed[sl].reshape(TOK, DIN)
        idxs = np.full(TOKP, -1.0, dtype=np.float32)
        idxs[:TOK] = idx[sl].reshape(TOK).astype(np.float32)
        xsT = np.ascontiguousarray(xs.T)                        # [768, 640]
        edT = np.ascontiguousarray(eds.T)
        # edH[p, i*1280 + h*640 + t] = ed.T[(2i+h)*128+p, t]
        edH = np.ascontiguousarray(
            edT.reshape(KP, 2, P, TOKP).transpose(2, 0, 1, 3)
            .reshape(P, KP * 2 * TOKP).astype(fp8))
        # x8H[p, ti*256 + h*128 + u] = x.T[h*128+p, ti*128+u]
        x8H = np.ascontiguousarray(
            xsT[:KF * P].reshape(KF, P, NT, P).transpose(1, 2, 0, 3)
            .reshape(P, NT * KF * P).astype(fp8))
        in_maps.append({
            "xT": np.ascontiguousarray(xsT.astype(bf16)),
            "x8H": x8H,
            "edH": edH,
            "idxf": np.ascontiguousarray(idxs.reshape(1, TOKP)),
            "cvals": cvals,
            "WT": WT,
            "W8H": W8H,
            "ldH": ldH,
            "luH": luH,
        })
    return in_maps


def kernel(x, edit_direction, concept_idx, lora_down, lora_up, W, b_lin,
           _trace=False, **_ignored):
    nc = get_bass()
    in_maps = make_in_maps(x, edit_direction, concept_idx, lora_down, lora_up,
                           W, b_lin)
    res = run_bass_kernel_spmd(nc, in_maps, core_ids=list(range(N_CORES)),
                               trace=_trace)
    out = np.concatenate([np.asarray(r["out"][:TOK], dtype=np.float32)
                          for r in res.results], axis=0)
    out = out.reshape(B, T, DOUT)
    if _trace:
        kernel.last_results = res
    return out
